# revision 8
# baseline (speedup 1.0000x reference)
"""Causal linear attention (elu+1 feature map) on 8 Trainium2 NeuronCores.

Full inputs (n=2, l=2048, h=8, d=64) fp32 are sharded over the 16 (n,h)
head-sequences: core i handles pairs (2i, 2i+1). Single-pass design (all 16
chunks of C=128 in one group), stride-2 scan (2 chunks per step).

Per step (chunks c0=2s, c1=2s+1):
  AT(c0), AT(c1): one matmul each (stationary kfT chunk, moving blocked qfb
  both pairs) into one PSUM bank [128, 512] = [ATc0p0|ATc0p1|ATc1p0|ATc1p1].
  Causal masking = ONE DVE multiply with a broadcast [128,128] tri mask
  (stride-0 AP over the 4 blocks).

  Cross-chunk term (c0 -> c1) never materializes a 128x128 AT block: the
  rank-64 factorization G_c0 = Kf_c0^T @ Vaug_c0 (the chunk's state
  increment, one extra matmul into its own PSUM bank + one 130-col ACT copy)
  gives cross = Qf_c1 @ G_c0 via the same stationary as Q@S.

  out_ps [128, 260] accumulates Q@S_snap + Q@G + ATm^T V per (chunk, pair),
  with a trailing denominator column from vaug's baked-in ones column.
  First matmul into each PSUM bank uses start=True (clears the bank; no
  zeros-init matmuls).

  Normalization is deferred: numerators are evacuated PSUM->SBUF stage by
  ScalarE (Copy), reciprocals collected per step into a [128, 32] tile by
  DVE, and every 2 steps one in-place f16 DVE multiply (recip broadcast via
  stride-0 AP) normalizes 512 staged columns, followed by the output DMA.

  The feature map f = min(exp(x), max(x+1, 1)) = elu(x)+1 runs once on qT
  and once on kT (exp on ScalarE, the rest on DVE); the natural-layout kf
  needed as the G/S stationary comes from an SBUF->SBUF DMA xbar transpose
  of the *feature-mapped* kfT (3D out AP [i, c, pd]), and the blocked qfb
  (off-pair blocks zero, for the shared-stationary AT trick) is assembled
  by two partition-sliced SBUF->SBUF DMAs from the dense qf plus one-time
  GpSimd memsets of the off-blocks.

Host layouts (fp16, all DMAs contiguous):
  qT, kT: (128, 2048)  [(64p + d), (128c + i)]   (host-transposed)
  v     : (128, 2080)  [i, (c, p, dv)] dv=65, ones baked in at dv=64
  o     : (128, 2048)  [i', (c, p, d)] fp16
"""
import numpy as np
from contextlib import ExitStack

import concourse.bacc as bacc
import concourse.bass as bass
import concourse.tile as tile
from concourse import mybir
from concourse.bass_utils import run_bass_kernel_spmd

N, L, H, D = 2, 2048, 8, 64
C = 128                 # chunk length
NCH = L // C            # 16 chunks
PAIRS = 2
W = NCH * PAIRS * D     # 2048
DV = D + 1              # 65: value cols + denominator ones col
VW = NCH * PAIRS * DV   # 2080
SW = PAIRS * DV         # 130

f16 = mybir.dt.float16
f32 = mybir.dt.float32
AF = mybir.ActivationFunctionType
OP = mybir.AluOpType


def build_kernel():
    nc = bacc.Bacc("TRN2", target_bir_lowering=False, debug=False, num_devices=8)
    qT_d = nc.dram_tensor("qT", (C, W), f16, kind="ExternalInput").ap()
    kT_d = nc.dram_tensor("kT", (C, W), f16, kind="ExternalInput").ap()
    v_d = nc.dram_tensor("v", (C, VW), f16, kind="ExternalInput").ap()
    o_d = nc.dram_tensor("o", (C, W), f16, kind="ExternalOutput").ap()

    with tile.TileContext(nc) as tc, ExitStack() as ctx:
        consts = ctx.enter_context(tc.tile_pool(name="consts", bufs=1))
        sm_pool = ctx.enter_context(tc.tile_pool(name="sm", bufs=2))
        atm_pool = ctx.enter_context(tc.tile_pool(name="atm", bufs=3))
        at_psum = ctx.enter_context(tc.tile_pool(name="at", bufs=2, space="PSUM"))
        out_psum = ctx.enter_context(tc.tile_pool(name="out", bufs=2, space="PSUM"))
        s_psum = ctx.enter_context(tc.tile_pool(name="sp", bufs=1, space="PSUM"))

        qT_t = consts.tile([C, W], f16)
        kT_t = consts.tile([C, W], f16)
        v_t = consts.tile([C, VW], f16)
        HW_ = W // 2
        nc.gpsimd.dma_start(v_t, v_d)
        nc.sync.dma_start(kT_t[:, 0:HW_], kT_d[:, 0:HW_])
        nc.sync.dma_start(kT_t[:, HW_:W], kT_d[:, HW_:W])
        nc.sync.dma_start(qT_t[:, 0:HW_], qT_d[:, 0:HW_])
        nc.sync.dma_start(qT_t[:, HW_:W], qT_d[:, HW_:W])

        # one-time consts: causal tri mask + qfb off-pair zero blocks
        tri = consts.tile([C, C], f16)
        nc.gpsimd.memset(tri, 0.0)
        nc.gpsimd.affine_select(
            out=tri, in_=tri, compare_op=OP.is_gt, fill=1.0,
            base=0, pattern=[[-1, C]], channel_multiplier=1,
        )
        qfb = consts.tile([C, PAIRS * W], f16)
        nc.gpsimd.memset(qfb[D:C, 0:W], 0.0)
        nc.gpsimd.memset(qfb[0:D, W:2 * W], 0.0)

        # feature maps f = min(exp(x), max(x+1, 1)), split in halves to
        # pipeline ACT (exp) / DVE (t, min) / DMA (transpose)
        eq = consts.tile([C, W], f16)
        tq = consts.tile([C, W], f16)
        ek = consts.tile([C, W], f16)
        tk = consts.tile([C, W], f16)
        kfT = consts.tile([C, W], f16)
        kf = consts.tile([C, W], f16)
        kf3 = kf.rearrange("i (c pd) -> i c pd", c=NCH)
        for h in range(2):
            hs = slice(h * HW_, (h + 1) * HW_)
            nc.scalar.activation(ek[:, hs], kT_t[:, hs], AF.Exp)
            nc.vector.tensor_scalar(out=tk[:, hs], in0=kT_t[:, hs],
                                    scalar1=1.0, scalar2=1.0,
                                    op0=OP.add, op1=OP.max)
            nc.vector.tensor_tensor(out=kfT[:, hs], in0=ek[:, hs],
                                    in1=tk[:, hs], op=OP.min)
            # natural-layout kf[i, (c, pd)] via SBUF->SBUF xbar transpose
            nc.sync.dma_start(kf3[:, h * NCH // 2:(h + 1) * NCH // 2],
                              kfT[:, hs], transpose=True)

        def fmap_q_half(h):
            hs = slice(h * HW_, (h + 1) * HW_)
            nc.scalar.activation(eq[:, hs], qT_t[:, hs], AF.Exp)
            nc.vector.tensor_scalar(out=tq[:, hs], in0=qT_t[:, hs],
                                    scalar1=1.0, scalar2=1.0,
                                    op0=OP.add, op1=OP.max)
            # blocked qfb written directly (off-pair blocks stay memset-zero)
            nc.vector.tensor_tensor(out=qfb[0:D, hs], in0=eq[0:D, hs],
                                    in1=tq[0:D, hs], op=OP.min)
            nc.vector.tensor_tensor(
                out=qfb[D:C, W + h * HW_:W + (h + 1) * HW_],
                in0=eq[D:C, hs], in1=tq[D:C, hs], op=OP.min)

        fmap_q_half(0)
        qfb4 = qfb.rearrange("r (p x) -> r p x", p=PAIRS)

        # running state S + per-step G in one 3-bank PSUM region:
        # bank 0 = S, banks 1/2 = G (rotating) -- lets one ACT copy per step
        # snapshot both S and G with a single strided read
        SG_full = s_psum.tile([C, 2 * 512], f32)
        S_ps = SG_full[:, 0:SW]

        stage = consts.tile([C, W], f16)
        recip_sb = consts.tile([C, 2 * NCH], f32)

        tri_b = bass.AP(
            tensor=tri.tensor, offset=tri.offset,
            ap=[list(tri.ap[0]), [0, 4], [1, C]],
        )

        for s in range(NCH // 2):
            c0, c1 = 2 * s, 2 * s + 1
            t0 = slice(c0 * C, (c0 + 1) * C)
            t1 = slice(c1 * C, (c1 + 1) * C)

            # AT for both chunks, both pairs: one PSUM bank [128, 512]
            at_ps = at_psum.tile([C, 4 * C], f32, tag="at")
            nc.tensor.matmul(at_ps[:, 0:2 * C], kfT[:, t0], qfb4[:, :, t0],
                             start=True, stop=False, skip_group_check=True)
            nc.tensor.matmul(at_ps[:, 2 * C:4 * C], kfT[:, t1], qfb4[:, :, t1],
                             start=False, stop=True, skip_group_check=True)

            # G_c0 = Kf_c0^T Vaug_c0 (cross source; also the S increment)
            goff = 512
            g_ps = SG_full[:, goff:goff + SW]
            nc.tensor.matmul(g_ps, kf[:, t0], v_t[:, c0 * SW:(c0 + 1) * SW],
                             start=True, stop=True, skip_group_check=True)
            # one ACT copy snapshots S (pre-update) and G together
            SG_sb = sm_pool.tile([C, 2 * SW], f16, tag="sg_sb")
            S_sb = SG_sb[:, 0:SW]
            G_sb = SG_sb[:, SW:2 * SW]
            if s == 0:
                nc.scalar.copy(G_sb, g_ps)   # S not yet written
            else:
                sg_src = bass.AP(
                    tensor=SG_full.tensor, offset=SG_full.offset,
                    ap=[list(SG_full.ap[0]), [goff, 2], [1, SW]],
                )
                sg_dst = SG_sb.rearrange("i (b x) -> i b x", b=2)
                nc.scalar.activation(sg_dst, sg_src, AF.Copy)

            # mask all 4 tri blocks in one DVE op (broadcast tri)
            atm = atm_pool.tile([C, 4 * C], f16, tag="atm")
            at3 = at_ps.rearrange("i (b x) -> i b x", b=4)
            atm3 = atm.rearrange("i (b x) -> i b x", b=4)
            nc.vector.tensor_tensor(out=atm3, in0=at3, in1=tri_b, op=OP.mult)

            # out accumulation [c0p0 | c0p1 | c1p0 | c1p1] (65 cols each);
            # even/odd steps share one double-bank tile so recip and the
            # normalize multiply batch over 2 steps
            if s % 2 == 0:
                out2 = out_psum.tile([C, 1024], f32, tag="out")
            ob_off = 512 * (s % 2)
            out_ps = out2[:, ob_off:ob_off + 4 * DV]
            first = [True]

            def omm(lhsT, rhs, blk, stop=False):
                nc.tensor.matmul(out_ps[:, blk * DV:(blk + 1) * DV], lhsT, rhs,
                                 start=first[0], stop=stop,
                                 skip_group_check=True)
                first[0] = False

            for p in range(PAIRS):
                vs = slice(p * DV, (p + 1) * DV)
                if s > 0:
                    omm(qfb[:, p * W + t0.start:p * W + t0.stop], S_sb[:, vs], p)
                    omm(qfb[:, p * W + t1.start:p * W + t1.stop], S_sb[:, vs],
                        2 + p)
                omm(qfb[:, p * W + t1.start:p * W + t1.stop], G_sb[:, vs], 2 + p)
            for p in range(PAIRS):
                nv0 = slice(c0 * SW + p * DV, c0 * SW + (p + 1) * DV)
                nv1 = slice(c1 * SW + p * DV, c1 * SW + (p + 1) * DV)
                omm(atm[:, p * C:(p + 1) * C], v_t[:, nv0], p)
                omm(atm[:, (2 + p) * C:(3 + p) * C], v_t[:, nv1], 2 + p,
                    stop=(p == PAIRS - 1))

            # S updates AFTER the out-MMs: they only must precede the NEXT
            # step's combined S|G snapshot (keeps PE from stalling on it)
            nc.tensor.matmul(S_ps, kf[:, t0], v_t[:, c0 * SW:(c0 + 1) * SW],
                             start=(s == 0), stop=False, skip_group_check=True)
            nc.tensor.matmul(S_ps, kf[:, t1], v_t[:, c1 * SW:(c1 + 1) * SW],
                             start=False, stop=(s == NCH // 2 - 1),
                             skip_group_check=True)

            if s == 1:
                fmap_q_half(1)

            # numerators -> stage (ACT); dens -> reciprocals every 2 steps
            num = out_ps.rearrange(
                "i (c p dv) -> i c p dv", c=2, p=PAIRS)[:, :, :, 0:D]
            st4 = stage.rearrange(
                "i (c p d) -> i c p d", c=NCH, p=PAIRS)[:, c0:c0 + 2]
            nc.scalar.activation(st4, num, AF.Copy)
            if s % 2 == 1:
                den = bass.AP(
                    tensor=out2.tensor, offset=out2.offset + D,
                    ap=[list(out2.ap[0]), [512, 2], [DV, 4]],
                )
                rout = recip_sb[:, 8 * (s // 2):8 * (s // 2 + 1)].rearrange(
                    "i (a b) -> i a b", a=2)
                nc.vector.reciprocal(rout, den)

            # deferred normalization + output DMA every 2 steps
            if s % 2 == 1:
                g4 = slice((s - 1) * 2 * C, (s + 1) * 2 * C)
                stg = stage[:, g4].rearrange(
                    "i (c p d) -> i c p d", c=4, p=PAIRS)
                rsl = recip_sb[:, 8 * (s // 2):8 * (s // 2 + 1)]
                rec_b = bass.AP(
                    tensor=rsl.tensor, offset=rsl.offset,
                    ap=[list(rsl.ap[0]), [2, 4], [1, 2], [0, D]],
                )
                nc.vector.tensor_tensor(out=stg, in0=stg, in1=rec_b,
                                        op=OP.mult)
                nc.sync.dma_start(o_d[:, g4], stage[:, g4])

    nc.compile()
    return nc


_nc_cache = None


def _get_nc():
    global _nc_cache
    if _nc_cache is None:
        _nc_cache = build_kernel()
    return _nc_cache


def _core_pairs(x, core):
    flat = x.transpose(0, 2, 1, 3).reshape(N * H, L, D)
    return flat[2 * core:2 * core + 2]          # (2, L, D) fp32


def _t_layout(xc):
    # (2, L, D) -> (128, 2048) [(64p + d), (128c + i)]
    return np.ascontiguousarray(
        xc.reshape(PAIRS, NCH, C, D).transpose(0, 3, 1, 2).reshape(C, W)
    ).astype(np.float16)


def _v_layout(xc):
    # (2, L, D) -> (128, 2080) [i, (c, p, dv)] with ones at dv=64
    v4 = xc.reshape(PAIRS, NCH, C, D).transpose(2, 1, 0, 3)  # (C, NCH, PAIRS, D)
    vaug = np.concatenate(
        [v4, np.ones((C, NCH, PAIRS, 1), v4.dtype)], axis=-1)
    return np.ascontiguousarray(vaug.reshape(C, VW)).astype(np.float16)


def make_in_maps(queries, keys, values):
    in_maps = []
    for core in range(8):
        in_maps.append({
            "qT": _t_layout(_core_pairs(queries, core)),
            "kT": _t_layout(_core_pairs(keys, core)),
            "v": _v_layout(_core_pairs(values, core)),
        })
    return in_maps


def _unpack_out(o):
    # (128, 2048) [i', (c, p, d)] -> (PAIRS, L, D)
    return o.reshape(C, NCH, PAIRS, D).transpose(2, 1, 0, 3).reshape(PAIRS, L, D)


def kernel(queries, keys, values):
    nc = _get_nc()
    in_maps = make_in_maps(queries, keys, values)
    res = run_bass_kernel_spmd(nc, in_maps, core_ids=list(range(8)))
    out = np.zeros((N, L, H, D), np.float32)
    for core in range(8):
        oc = _unpack_out(res.results[core]["o"].astype(np.float32))
        for p in range(PAIRS):
            flat = 2 * core + p
            out[flat // H, :, flat % H, :] = oc[p]
    return out
